# revision 4
# baseline (speedup 1.0000x reference)
"""Multi-head causal attention (B=4, S=2048, D=1024, H=16) on 8 TRN2 NeuronCores.

Sharding: core c -> (batch b = c//2, head-group g = c%2). Each core computes
8 heads for one batch: QKV projection (tensor-parallel column slice), causal
softmax attention, and a row-parallel slice of the output projection. The two
cores of a batch produce partial outputs that the host sums; biases that
commute with the attention (v bias, out bias) are folded into a single
host-side vector add.

v2 design (fused schedule, bf16):
 - All activations/weights are bf16 (PSUM accumulation stays fp32); rel-err
   budget is 2e-2, bf16 keeps us ~2 orders below that. Matmuls run at 1
   cycle/row at any width (no fp32r <256-wide penalty), SBUF/DMA traffic
   halves.
 - The attention inner loop is Scalar(ACT)-bound: exp on [128,w] costs
   ~0.83ns/elem + ~370ns/instr while its two surrounding matmuls cost only
   ~426ns. So QKV-projection and output-projection matmul chunks are
   interleaved INTO the attention stream as tensor-engine filler: window it
   runs attention(it) heads + qkv(it+1) chunks + proj(it-1) chunks. This
   keeps the tensor queue dense (it never waits on exp), which also keeps
   the PE p-state at max clock (2.4 GHz needs >3us of continuous busy).
 - Key (padding) mask is applied to the V tiles instead of an exp bias:
   zeroing vx row k (both the v values and the denominator-ones column)
   removes key k from numerator and denominator exactly.
 - Softmax denominators ride as a 65th ones-column in the packed V tiles;
   normalization is reciprocal_approx_fast (DVE) + partition_broadcast
   (GpSimd) + one multiply, off the tensor critical path (per-head PSUM
   accumulators, 3 in flight).
 - PSUM: 3 banks scores pipeline + 3 banks attn@V accumulators + 2 banks
   qkv/proj chunks = 8.
"""

import numpy as np
from contextlib import ExitStack

B, S, D, H = 4, 2048, 1024, 16
HD = D // H          # 64
HPC = H // 2         # 8 heads per core
DV = HPC * HD        # 512 v-dims per core
N_CORES = 8
SB = 512             # i-tile width (matmul N)
NSB = S // SB        # 4
NJT = S // 128       # 16 j-tiles

_CACHE = {}


def _build_module():
    import os
    KREP = int(os.environ.get("KREP", "1"))
    SPSB = int(os.environ.get("SPSB", "3"))
    APSB = int(os.environ.get("APSB", "3"))
    GPSB = int(os.environ.get("GPSB", "2"))
    EPB = int(os.environ.get("EPB", "6"))
    XPB = int(os.environ.get("XPB", "16"))
    NPB = int(os.environ.get("NPB", "8"))
    YPB = int(os.environ.get("YPB", "4"))
    FILL_EVERY = int(os.environ.get("FILL_EVERY", "8"))
    import concourse.bacc as bacc
    import concourse.mybir as mybir
    import concourse.tile as tile
    from concourse._compat import get_trn_type

    F32 = mybir.dt.float32
    BF16 = mybir.dt.bfloat16
    EXP = mybir.ActivationFunctionType.Exp

    nc = bacc.Bacc(get_trn_type() or "TRN2", target_bir_lowering=False, debug=False)

    # ---- DRAM parameters (per core) ----
    xT = nc.declare_dram_parameter("xT", [D, S], BF16, isOutput=False)       # x[b].T
    wq = nc.declare_dram_parameter("wq", [D, DV], BF16, isOutput=False)      # (W_q,g / 8).T
    wk = nc.declare_dram_parameter("wk", [D, DV], BF16, isOutput=False)      # W_k,g.T
    wv = nc.declare_dram_parameter("wv", [D, DV], BF16, isOutput=False)      # W_v,g.T
    ow = nc.declare_dram_parameter("ow", [DV, D], BF16, isOutput=False)      # W_out[:, g].T
    bq = nc.declare_dram_parameter("bq", [DV, 1], F32, isOutput=False)       # q bias / 8
    bk = nc.declare_dram_parameter("bk", [DV, 1], F32, isOutput=False)
    km = nc.declare_dram_parameter("km", [S, 1], F32, isOutput=False)        # key mask 0/1
    y = nc.declare_dram_parameter("y", [S, D], BF16, isOutput=True)          # partial output

    with tile.TileContext(nc) as tc, ExitStack() as octx:
        # ---- persistent SBUF ----
        pers = octx.enter_context(tc.tile_pool(name="pers", bufs=1))
        qT = [pers.tile([128, S], BF16, tag=f"qT{p}", name=f"qT{p}") for p in range(4)]
        kT = [pers.tile([128, S], BF16, tag=f"kT{p}", name=f"kT{p}") for p in range(4)]
        vx = [pers.tile([128, HPC * 65], BF16, tag=f"vx{j}", name=f"vx{j}") for j in range(NJT)]
        anT = [pers.tile([128, S], BF16, tag=f"anT{p}", name=f"anT{p}") for p in range(4)]
        bq_t = pers.tile([128, 4], F32, tag="bq")
        bk_t = pers.tile([128, 4], F32, tag="bk")
        km_t = pers.tile([128, NJT], F32, tag="km")
        cmt = pers.tile([128, 128], F32, tag="cmt")   # triangular boundary mask
        ones8 = pers.tile([128, HPC], BF16, tag="ones8")

        nc.sync.dma_start(bq_t[:], bq[:].squeeze(1).rearrange("(t p) -> p t", p=128))
        nc.sync.dma_start(bk_t[:], bk[:].squeeze(1).rearrange("(t p) -> p t", p=128))
        nc.sync.dma_start(km_t[:], km[:].squeeze(1).rearrange("(t p) -> p t", p=128))

        nc.vector.memset(ones8[:], 1.0)
        # keep (0) iff c - pj >= 0, else -1e30  (boundary block: col c = local
        # query offset, partition pj = key offset within the diagonal block)
        nc.vector.memset(cmt[:], 0.0)
        nc.gpsimd.affine_select(
            out=cmt[:], in_=cmt[:], compare_op=mybir.AluOpType.is_ge,
            fill=-1e30, base=0, pattern=[[1, 128]], channel_multiplier=-1,
        )

        for _rep in range(KREP):
            with ExitStack() as ctx:
                wpool = ctx.enter_context(tc.tile_pool(name="wpool", bufs=1))
                wq_t = [wpool.tile([128, DV], BF16, tag=f"wq{d}", name=f"wq{d}") for d in range(8)]
                wk_t = [wpool.tile([128, DV], BF16, tag=f"wk{d}", name=f"wk{d}") for d in range(8)]
                wv_t = [wpool.tile([128, DV], BF16, tag=f"wv{d}", name=f"wv{d}") for d in range(8)]
                ow_t = [wpool.tile([128, SB], BF16, tag=f"ow{i}", name=f"ow{i}") for i in range(8)]

                xpool = ctx.enter_context(tc.tile_pool(name="xpool", bufs=XPB))
                gps = ctx.enter_context(tc.tile_pool(name="gps", bufs=GPSB, space="PSUM"))
                sps = ctx.enter_context(tc.tile_pool(name="sps", bufs=SPSB, space="PSUM"))
                aps = ctx.enter_context(tc.tile_pool(name="aps", bufs=APSB, space="PSUM"))
                epool = ctx.enter_context(tc.tile_pool(name="epool", bufs=EPB))
                npool = ctx.enter_context(tc.tile_pool(name="npool", bufs=NPB))
                ypool = ctx.enter_context(tc.tile_pool(name="ypool", bufs=YPB))

                # wq first: the cold-start qkv(0) q-chunks only need wq + xt(0)
                for d in range(8):
                    nc.sync.dma_start(wq_t[d][:], wq[128 * d:128 * d + 128, :])

                xt_tiles = {}

                def emit_xt_dma(sblk):
                    ssl = slice(SB * sblk, SB * sblk + SB)
                    xt = []
                    for d in range(8):
                        t = xpool.tile([128, SB], BF16, tag="xt")
                        nc.sync.dma_start(t[:], xT[128 * d:128 * d + 128, ssl])
                        xt.append(t)
                    xt_tiles[sblk] = xt

                def qkv_chunks(sblk):
                    ssl = slice(SB * sblk, SB * sblk + SB)
                    chunks = []
                    for wt, bt, dst in ((wq_t, bq_t, qT), (wk_t, bk_t, kT)):
                        for o in range(4):
                            def c(wt=wt, bt=bt, dst=dst, o=o, sblk=sblk, ssl=ssl):
                                xt = xt_tiles[sblk]
                                osl = slice(128 * o, 128 * o + 128)
                                ps = gps.tile([128, SB], F32, tag="ps")
                                for d in range(8):
                                    nc.tensor.matmul(ps[:], wt[d][:, osl], xt[d][:],
                                                     start=(d == 0), stop=(d == 7))
                                nc.vector.tensor_scalar_add(dst[o][:, ssl], ps[:], bt[:, o:o + 1])
                            chunks.append(c)
                    for ssub in range(4):
                        def c(ssub=ssub, sblk=sblk):
                            jt = 4 * sblk + ssub
                            xt = xt_tiles[sblk]
                            ps = gps.tile([128, SB], F32, tag="ps")
                            for d in range(8):
                                nc.tensor.matmul(ps[:], xt[d][:, 128 * ssub:128 * ssub + 128],
                                                 wv_t[d][:], start=(d == 0), stop=(d == 7))
                            # masked v write: zero vx rows of masked keys
                            dst = vx[jt][:].rearrange("p (h c) -> p h c", c=65)[:, :, 0:64]
                            src = ps[:].rearrange("p (h c) -> p h c", c=64)
                            nc.vector.tensor_scalar_mul(dst, src, km_t[:, jt:jt + 1])
                            ones_view = vx[jt][:].rearrange("p (h c) -> p h c", c=65)[:, :, 64:65]
                            nc.vector.tensor_scalar_mul(
                                ones_view, ones8[:].rearrange("p (h c) -> p h c", c=1),
                                km_t[:, jt:jt + 1])
                        chunks.append(c)
                    return chunks

                def proj_chunks(it):
                    chunks = []
                    for st in range(4 * it, 4 * it + 4):
                        for ot in range(2):
                            def c(st=st, ot=ot):
                                ssl = slice(128 * st, 128 * st + 128)
                                ps = gps.tile([128, SB], F32, tag="ps")
                                for p4 in range(4):
                                    nc.tensor.matmul(ps[:], anT[p4][:, ssl], ow_t[2 * p4 + ot][:],
                                                     start=(p4 == 0), stop=(p4 == 3))
                                yt = ypool.tile([128, SB], BF16, tag="yt")
                                nc.vector.tensor_copy(yt[:], ps[:])
                                nc.sync.dma_start(y[ssl, SB * ot:SB * ot + SB], yt[:])
                            chunks.append(c)
                    return chunks

                filler = []
                jcount = [0]

                def tick_filler():
                    jcount[0] += 1
                    if jcount[0] % FILL_EVERY == 0 and filler:
                        filler.pop(0)()

                def attn_head(it, h):
                    p, half = divmod(h, 2)
                    P = slice(64 * half, 64 * half + 64)
                    i0 = SB * it
                    njt = 4 * it + 4
                    pa = aps.tile([65, SB], F32, tag="pa")
                    pend = []
                    for jt in range(njt):
                        r = jt - 4 * it          # >=0: diagonal j-tile
                        c0 = 128 * r if r > 0 else 0
                        jsl = slice(128 * jt, 128 * jt + 128)
                        s = sps.tile([128, SB - c0], F32, tag="s")
                        nc.tensor.matmul(s[:], kT[p][P, jsl], qT[p][P, i0 + c0:i0 + SB],
                                         start=True, stop=True)
                        if r >= 0:  # triangular boundary block at local cols 0:128
                            nc.vector.tensor_add(s[:, 0:128], s[:, 0:128], cmt[:])
                        e = epool.tile([128, SB], BF16, tag="e")
                        if c0 > 0:
                            nc.gpsimd.memset(e[:, 0:c0], 0.0)
                        nc.scalar.activation(e[:, c0:SB], s[:], EXP)
                        pend.append((jt, e))
                        if len(pend) > 2:
                            j0, e0 = pend.pop(0)
                            nc.tensor.matmul(pa[:], vx[j0][:, 65 * h:65 * h + 65], e0[:],
                                             start=(j0 == 0), stop=(j0 == njt - 1))
                        tick_filler()
                    for j0, e0 in pend:
                        nc.tensor.matmul(pa[:], vx[j0][:, 65 * h:65 * h + 65], e0[:],
                                         start=(j0 == 0), stop=(j0 == njt - 1))
                    rec = npool.tile([1, SB], F32, tag="rec")
                    nc.vector.reciprocal(rec[:], pa[64:65, :])
                    rb = npool.tile([64, SB], F32, tag="rb")
                    nc.gpsimd.partition_broadcast(rb[:], rec[:])
                    nc.vector.tensor_mul(anT[p][P, i0:i0 + SB], pa[0:64, :], rb[:])

                # ---- schedule ----
                emit_xt_dma(0)
                cold = qkv_chunks(0)
                # q chunks (need only wq) first, then load remaining weights
                for c in cold[0:4]:
                    c()
                for d in range(8):
                    nc.sync.dma_start(wk_t[d][:], wk[128 * d:128 * d + 128, :])
                for d in range(8):
                    nc.sync.dma_start(wv_t[d][:], wv[128 * d:128 * d + 128, :])
                for p in range(4):
                    for ot in range(2):
                        nc.sync.dma_start(ow_t[2 * p + ot][:],
                                          ow[128 * p:128 * p + 128, SB * ot:SB * ot + SB])
                for c in cold[4:]:
                    c()

                for it in range(NSB):
                    if it + 1 < NSB:
                        emit_xt_dma(it + 1)
                        filler += qkv_chunks(it + 1)
                    if it >= 1:
                        filler += proj_chunks(it - 1)
                    for h in range(HPC):
                        attn_head(it, h)
                    while filler:   # prerequisites for the next window
                        filler.pop(0)()
                for c in proj_chunks(NSB - 1):
                    c()

    nc.compile()
    return nc


def _get_module():
    if "nc" not in _CACHE:
        _CACHE["nc"] = _build_module()
    return _CACHE["nc"]


def _host_prep(x, mask, qkv_w, qkv_b, out_w):
    """Per-core input maps."""
    import ml_dtypes
    bf16 = ml_dtypes.bfloat16
    scale = np.float32(1.0 / np.sqrt(HD))
    in_maps = []
    for c in range(N_CORES):
        b, g = divmod(c, 2)
        qr = slice(g * DV, g * DV + DV)
        kr = slice(D + g * DV, D + g * DV + DV)
        vr = slice(2 * D + g * DV, 2 * D + g * DV + DV)
        in_maps.append({
            "xT": np.ascontiguousarray(x[b].T).astype(bf16),
            "wq": np.ascontiguousarray(qkv_w[qr].T * scale).astype(bf16),
            "wk": np.ascontiguousarray(qkv_w[kr].T).astype(bf16),
            "wv": np.ascontiguousarray(qkv_w[vr].T).astype(bf16),
            "ow": np.ascontiguousarray(out_w[:, g * DV:g * DV + DV].T).astype(bf16),
            "bq": (qkv_b[qr] * scale).reshape(DV, 1).astype(np.float32),
            "bk": qkv_b[kr].reshape(DV, 1).astype(np.float32),
            "km": (mask[b] != 0).astype(np.float32).reshape(S, 1),
        })
    return in_maps


def _host_gather(results, qkv_b, out_b, out_w):
    # constant bias: out_b + W_out @ v_bias (v bias commutes through attention)
    bias = (out_b + out_w @ qkv_b[2 * D:3 * D]).astype(np.float32)
    y = np.empty((B, S, D), dtype=np.float32)
    for b in range(B):
        y[b] = (results[2 * b]["y"].astype(np.float32)
                + results[2 * b + 1]["y"].astype(np.float32)
                + bias[None, :])
    return y


def kernel(x, mask, qkv_w, qkv_b, out_w, out_b):
    import time
    from concourse.bass_utils import run_bass_kernel_spmd

    nc = _get_module()
    in_maps = _host_prep(x, mask, qkv_w, qkv_b, out_w)
    last = None
    for attempt in range(3):
        try:
            res = run_bass_kernel_spmd(nc, in_maps, core_ids=list(range(N_CORES)))
            return _host_gather(res.results, qkv_b, out_b, out_w)
        except Exception as e:  # rare transient device faults: retry after recovery
            last = e
            time.sleep(10 * (attempt + 1))
    raise last


# revision 9
# speedup vs baseline: 2.4665x; 2.4665x over previous
"""Multi-head causal attention (B=4, S=2048, D=1024, H=16) on 8 TRN2 NeuronCores.

Sharding: core c -> (batch b = c//2, head-group g = c%2). Each core computes
8 heads for one batch: QKV projection (tensor-parallel column slice), causal
softmax attention, and a row-parallel slice of the output projection. The two
cores of a batch produce partial outputs that the host sums; biases that
commute with the attention (v bias, out bias) are folded into a single
host-side vector add.

v3 design (fused schedule, bf16, paired score tiles):
 - All activations/weights are bf16 (PSUM accumulation stays fp32).
 - The attention inner loop is Scalar(ACT)-bound (exp ~0.83ns/elem +
   ~370ns/instr), so QKV-projection and output-projection matmul chunks are
   interleaved INTO the attention stream as tensor-engine filler. This keeps
   the tensor queue dense, which also keeps the PE p-state at max clock.
 - Scores for two consecutive key j-tiles go into ONE [128,1024] PSUM
   supertile (2 banks); a single exp activation covers both, halving the
   per-instruction Scalar overhead. The gap strip between the two valid
   regions of a diagonal pair is exp'd from uninitialized PSUM but never
   read (attn@V is causally trimmed).
 - attn@V matmuls cover only the causally valid query range [c0:512]; the
   j-tile-0 matmul (always full width) initializes the accumulator.
 - Key (padding) mask is applied to the V tiles instead of an exp bias:
   zeroing vx row k (v values and the denominator-ones column) removes key
   k from numerator and denominator exactly.
 - Normalization: per head only two cheap copies (denominator row -> den8,
   unnormalized psum -> anT) so the PSUM accumulator frees in ~1.3us
   (aps=2 banks suffice); per window one batched reciprocal [8,512] and 8
   broadcast+multiply ops, all off the tensor critical path.
 - PSUM: 2x2 banks scores pairs + 2 attn@V accumulators + 2 qkv/proj = 8.
"""

import numpy as np
from contextlib import ExitStack

B, S, D, H = 4, 2048, 1024, 16
HD = D // H          # 64
HPC = H // 2         # 8 heads per core
DV = HPC * HD        # 512 v-dims per core
N_CORES = 8
SB = 512             # i-tile width (matmul N)
NSB = S // SB        # 4
NJT = S // 128       # 16 j-tiles

_CACHE = {}


def _build_module():
    import os
    KREP = int(os.environ.get("KREP", "1"))
    SPSB = int(os.environ.get("SPSB", "2"))   # pairs of banks
    APSB = int(os.environ.get("APSB", "2"))
    GPSB = int(os.environ.get("GPSB", "2"))
    EPB = int(os.environ.get("EPB", "3"))     # [128,1024] bf16 e tiles
    XPB = int(os.environ.get("XPB", "16"))
    NPB = int(os.environ.get("NPB", "4"))
    RBB = int(os.environ.get("RBB", "4"))
    YPB = int(os.environ.get("YPB", "4"))
    FILL_EVERY = int(os.environ.get("FILL_EVERY", "8"))
    import concourse.bacc as bacc
    import concourse.mybir as mybir
    import concourse.tile as tile
    from concourse._compat import get_trn_type

    F32 = mybir.dt.float32
    BF16 = mybir.dt.bfloat16
    EXP = mybir.ActivationFunctionType.Exp

    nc = bacc.Bacc(get_trn_type() or "TRN2", target_bir_lowering=False, debug=False)

    # ---- DRAM parameters (per core) ----
    xT = nc.declare_dram_parameter("xT", [D, S], BF16, isOutput=False)       # x[b].T
    wq = nc.declare_dram_parameter("wq", [D, DV], BF16, isOutput=False)      # (W_q,g / 8).T
    wk = nc.declare_dram_parameter("wk", [D, DV], BF16, isOutput=False)      # W_k,g.T
    wv = nc.declare_dram_parameter("wv", [D, DV], BF16, isOutput=False)      # W_v,g.T
    ow = nc.declare_dram_parameter("ow", [DV, D], BF16, isOutput=False)      # W_out[:, g].T
    bq = nc.declare_dram_parameter("bq", [DV, 1], F32, isOutput=False)       # q bias / 8
    bk = nc.declare_dram_parameter("bk", [DV, 1], F32, isOutput=False)
    km = nc.declare_dram_parameter("km", [S, 1], F32, isOutput=False)        # key mask 0/1
    y = nc.declare_dram_parameter("y", [S, D], BF16, isOutput=True)          # partial output

    with tile.TileContext(nc) as tc, ExitStack() as octx:
        # ---- persistent SBUF ----
        pers = octx.enter_context(tc.tile_pool(name="pers", bufs=1))
        qT = [pers.tile([128, S], BF16, tag=f"qT{p}", name=f"qT{p}") for p in range(4)]
        kT = [pers.tile([128, S], BF16, tag=f"kT{p}", name=f"kT{p}") for p in range(4)]
        vx = [pers.tile([128, HPC * 65], BF16, tag=f"vx{j}", name=f"vx{j}") for j in range(NJT)]
        anT = [pers.tile([128, S], BF16, tag=f"anT{p}", name=f"anT{p}") for p in range(4)]
        bq_t = pers.tile([128, 4], F32, tag="bq")
        bk_t = pers.tile([128, 4], F32, tag="bk")
        km_t = pers.tile([128, NJT], F32, tag="km")
        cmt = pers.tile([128, 128], F32, tag="cmt")   # triangular boundary mask
        ones8 = pers.tile([128, HPC], BF16, tag="ones8")

        nc.sync.dma_start(bq_t[:], bq[:].squeeze(1).rearrange("(t p) -> p t", p=128))
        nc.sync.dma_start(bk_t[:], bk[:].squeeze(1).rearrange("(t p) -> p t", p=128))
        nc.sync.dma_start(km_t[:], km[:].squeeze(1).rearrange("(t p) -> p t", p=128))

        nc.vector.memset(ones8[:], 1.0)
        # keep (0) iff c - pj >= 0, else -1e30  (boundary block: col c = local
        # query offset, partition pj = key offset within the diagonal block)
        nc.vector.memset(cmt[:], 0.0)
        nc.gpsimd.affine_select(
            out=cmt[:], in_=cmt[:], compare_op=mybir.AluOpType.is_ge,
            fill=-1e30, base=0, pattern=[[1, 128]], channel_multiplier=-1,
        )

        for _rep in range(KREP):
            with ExitStack() as ctx:
                wpool = ctx.enter_context(tc.tile_pool(name="wpool", bufs=1))
                wq_t = [wpool.tile([128, DV], BF16, tag=f"wq{d}", name=f"wq{d}") for d in range(8)]
                wk_t = [wpool.tile([128, DV], BF16, tag=f"wk{d}", name=f"wk{d}") for d in range(8)]
                wv_t = [wpool.tile([128, DV], BF16, tag=f"wv{d}", name=f"wv{d}") for d in range(8)]
                ow_t = [wpool.tile([128, SB], BF16, tag=f"ow{i}", name=f"ow{i}") for i in range(8)]

                xpool = ctx.enter_context(tc.tile_pool(name="xpool", bufs=XPB))
                gps = ctx.enter_context(tc.tile_pool(name="gps", bufs=GPSB, space="PSUM"))
                sps = ctx.enter_context(tc.tile_pool(name="sps", bufs=SPSB, space="PSUM"))
                aps = ctx.enter_context(tc.tile_pool(name="aps", bufs=APSB, space="PSUM"))
                epool = ctx.enter_context(tc.tile_pool(name="epool", bufs=EPB))
                npool = ctx.enter_context(tc.tile_pool(name="npool", bufs=NPB))
                rbp = ctx.enter_context(tc.tile_pool(name="rbp", bufs=RBB))
                ypool = ctx.enter_context(tc.tile_pool(name="ypool", bufs=YPB))

                # wq first: the cold-start qkv(0) q-chunks only need wq + xt(0)
                for d in range(8):
                    nc.sync.dma_start(wq_t[d][:], wq[128 * d:128 * d + 128, :])

                xt_tiles = {}

                def emit_xt_dma(sblk):
                    ssl = slice(SB * sblk, SB * sblk + SB)
                    xt = []
                    for d in range(8):
                        t = xpool.tile([128, SB], BF16, tag="xt")
                        nc.sync.dma_start(t[:], xT[128 * d:128 * d + 128, ssl])
                        xt.append(t)
                    xt_tiles[sblk] = xt

                def qkv_chunks(sblk):
                    ssl = slice(SB * sblk, SB * sblk + SB)
                    chunks = []
                    for wt, bt, dst in ((wq_t, bq_t, qT), (wk_t, bk_t, kT)):
                        for o in range(4):
                            def c(wt=wt, bt=bt, dst=dst, o=o, sblk=sblk, ssl=ssl):
                                xt = xt_tiles[sblk]
                                osl = slice(128 * o, 128 * o + 128)
                                ps = gps.tile([128, SB], F32, tag="ps")
                                for d in range(8):
                                    nc.tensor.matmul(ps[:], wt[d][:, osl], xt[d][:],
                                                     start=(d == 0), stop=(d == 7))
                                nc.vector.tensor_scalar_add(dst[o][:, ssl], ps[:], bt[:, o:o + 1])
                            chunks.append(c)
                    for ssub in range(4):
                        def c(ssub=ssub, sblk=sblk):
                            jt = 4 * sblk + ssub
                            xt = xt_tiles[sblk]
                            ps = gps.tile([128, SB], F32, tag="ps")
                            for d in range(8):
                                nc.tensor.matmul(ps[:], xt[d][:, 128 * ssub:128 * ssub + 128],
                                                 wv_t[d][:], start=(d == 0), stop=(d == 7))
                            # masked v write: zero vx rows of masked keys
                            dst = vx[jt][:].rearrange("p (h c) -> p h c", c=65)[:, :, 0:64]
                            src = ps[:].rearrange("p (h c) -> p h c", c=64)
                            nc.vector.tensor_scalar_mul(dst, src, km_t[:, jt:jt + 1])
                            ones_view = vx[jt][:].rearrange("p (h c) -> p h c", c=65)[:, :, 64:65]
                            nc.vector.tensor_scalar_mul(
                                ones_view, ones8[:].rearrange("p (h c) -> p h c", c=1),
                                km_t[:, jt:jt + 1])
                        chunks.append(c)
                    return chunks

                def proj_chunks(it):
                    chunks = []
                    for st in range(4 * it, 4 * it + 4):
                        for ot in range(2):
                            def c(st=st, ot=ot):
                                ssl = slice(128 * st, 128 * st + 128)
                                ps = gps.tile([128, SB], F32, tag="ps")
                                for p4 in range(4):
                                    nc.tensor.matmul(ps[:], anT[p4][:, ssl], ow_t[2 * p4 + ot][:],
                                                     start=(p4 == 0), stop=(p4 == 3))
                                yt = ypool.tile([128, SB], BF16, tag="yt")
                                nc.vector.tensor_copy(yt[:], ps[:])
                                nc.sync.dma_start(y[ssl, SB * ot:SB * ot + SB], yt[:])
                            chunks.append(c)
                    return chunks

                filler = []
                jcount = [0]

                def tick_filler():
                    jcount[0] += 1
                    if jcount[0] % FILL_EVERY == 0 and filler:
                        filler.pop(0)()

                def attn_head(it, h, dent):
                    p, half = divmod(h, 2)
                    P = slice(64 * half, 64 * half + 64)
                    i0 = SB * it
                    njt = 4 * it + 4
                    pa = aps.tile([65, SB], F32, tag="pa")

                    def attnv(jt, e2):
                        ehalf = jt % 2
                        r = jt - 4 * it
                        c0 = 128 * r if r > 0 else 0
                        nc.tensor.matmul(pa[:, c0:SB],
                                         vx[jt][:, 65 * h:65 * h + 65],
                                         e2[:, SB * ehalf + c0:SB * ehalf + SB],
                                         start=(jt == 0), stop=(jt == njt - 1))

                    prev = None
                    for jp in range(njt // 2):
                        jta, jtb = 2 * jp, 2 * jp + 1
                        ra, rb_ = jta - 4 * it, jtb - 4 * it
                        c0a = 128 * ra if ra > 0 else 0
                        c0b = 128 * rb_ if rb_ > 0 else 0
                        s2 = sps.tile([128, 2 * SB], F32, tag="s2")
                        nc.tensor.matmul(s2[:, c0a:SB], kT[p][P, 128 * jta:128 * jta + 128],
                                         qT[p][P, i0 + c0a:i0 + SB], start=True, stop=True)
                        nc.tensor.matmul(s2[:, SB + c0b:2 * SB], kT[p][P, 128 * jtb:128 * jtb + 128],
                                         qT[p][P, i0 + c0b:i0 + SB], start=True, stop=True)
                        if ra >= 0:  # triangular boundary blocks
                            nc.vector.tensor_add(s2[:, c0a:c0a + 128], s2[:, c0a:c0a + 128], cmt[:])
                        if rb_ >= 0:
                            nc.vector.tensor_add(s2[:, SB + c0b:SB + c0b + 128],
                                                 s2[:, SB + c0b:SB + c0b + 128], cmt[:])
                        e2 = epool.tile([128, 2 * SB], BF16, tag="e")
                        # one exp covers both j-tiles; the [SB, SB+c0b) strip is
                        # garbage from uninitialized PSUM but never read
                        nc.scalar.activation(e2[:, c0a:2 * SB], s2[:, c0a:2 * SB], EXP)
                        if prev is not None:
                            attnv(2 * prev[0], prev[1])
                            attnv(2 * prev[0] + 1, prev[1])
                        prev = (jp, e2)
                        tick_filler()
                        tick_filler()
                    attnv(2 * prev[0], prev[1])
                    attnv(2 * prev[0] + 1, prev[1])
                    # cheap copies so the PSUM accumulator frees fast
                    dr = 32 * (h % 4)
                    nc.vector.tensor_copy(dent[dr:dr + 1, :], pa[64:65, :])
                    nc.vector.tensor_copy(anT[p][P, i0:i0 + SB], pa[0:64, :])

                # ---- schedule ----
                emit_xt_dma(0)
                cold = qkv_chunks(0)
                # q chunks (need only wq) first, then load remaining weights
                for c in cold[0:4]:
                    c()
                for d in range(8):
                    nc.sync.dma_start(wk_t[d][:], wk[128 * d:128 * d + 128, :])
                for d in range(8):
                    nc.sync.dma_start(wv_t[d][:], wv[128 * d:128 * d + 128, :])
                for p in range(4):
                    for ot in range(2):
                        nc.sync.dma_start(ow_t[2 * p + ot][:],
                                          ow[128 * p:128 * p + 128, SB * ot:SB * ot + SB])
                for c in cold[4:]:
                    c()

                for it in range(NSB):
                    if it + 1 < NSB:
                        emit_xt_dma(it + 1)
                        filler += qkv_chunks(it + 1)
                    if it >= 1:
                        filler += proj_chunks(it - 1)
                    # 4 heads' denominator rows per [97,512] tile at
                    # partitions {0,32,64,96} (DVE partition bases must be
                    # 32-aligned); one reciprocal covers all four (cost is
                    # free-size-bound; untouched partitions are never read)
                    dents = [npool.tile([97, SB], F32, tag="dent", name=f"dent{it}_{t}") for t in range(2)]
                    for h in range(HPC):
                        attn_head(it, h, dents[h // 4])
                    dentr = [npool.tile([97, SB], F32, tag="dentr", name=f"dentr{it}_{t}") for t in range(2)]
                    for t in range(2):
                        nc.vector.reciprocal(dentr[t][:], dents[t][:])
                    i0 = SB * it
                    for h in range(HPC):
                        p, half = divmod(h, 2)
                        P = slice(64 * half, 64 * half + 64)
                        dr = 32 * (h % 4)
                        # partition_broadcast only honors base-0 in/out APs on
                        # HW: stage the reciprocal row at partition 0 first,
                        # broadcast to all 128, multiply base-aligned slices
                        src = dentr[h // 4][dr:dr + 1, :]
                        if dr != 0:
                            sd0 = rbp.tile([1, SB], F32, tag="sd0")
                            nc.vector.tensor_copy(sd0[:], src)
                            src = sd0[:]
                        rb = rbp.tile([128, SB], F32, tag="rb")
                        nc.gpsimd.partition_broadcast(rb[:], src)
                        nc.vector.tensor_mul(anT[p][P, i0:i0 + SB],
                                             anT[p][P, i0:i0 + SB], rb[P, :])
                    while filler:   # prerequisites for the next window
                        filler.pop(0)()
                for c in proj_chunks(NSB - 1):
                    c()

    nc.compile()
    return nc


def _get_module():
    if "nc" not in _CACHE:
        _CACHE["nc"] = _build_module()
    return _CACHE["nc"]


def _host_prep(x, mask, qkv_w, qkv_b, out_w):
    """Per-core input maps."""
    import ml_dtypes
    bf16 = ml_dtypes.bfloat16
    scale = np.float32(1.0 / np.sqrt(HD))
    in_maps = []
    for c in range(N_CORES):
        b, g = divmod(c, 2)
        qr = slice(g * DV, g * DV + DV)
        kr = slice(D + g * DV, D + g * DV + DV)
        vr = slice(2 * D + g * DV, 2 * D + g * DV + DV)
        in_maps.append({
            "xT": np.ascontiguousarray(x[b].T).astype(bf16),
            "wq": np.ascontiguousarray(qkv_w[qr].T * scale).astype(bf16),
            "wk": np.ascontiguousarray(qkv_w[kr].T).astype(bf16),
            "wv": np.ascontiguousarray(qkv_w[vr].T).astype(bf16),
            "ow": np.ascontiguousarray(out_w[:, g * DV:g * DV + DV].T).astype(bf16),
            "bq": (qkv_b[qr] * scale).reshape(DV, 1).astype(np.float32),
            "bk": qkv_b[kr].reshape(DV, 1).astype(np.float32),
            "km": (mask[b] != 0).astype(np.float32).reshape(S, 1),
        })
    return in_maps


def _host_gather(results, qkv_b, out_b, out_w):
    # constant bias: out_b + W_out @ v_bias (v bias commutes through attention)
    bias = (out_b + out_w @ qkv_b[2 * D:3 * D]).astype(np.float32)
    y = np.empty((B, S, D), dtype=np.float32)
    for b in range(B):
        y[b] = (results[2 * b]["y"].astype(np.float32)
                + results[2 * b + 1]["y"].astype(np.float32)
                + bias[None, :])
    return y


def kernel(x, mask, qkv_w, qkv_b, out_w, out_b):
    import time
    from concourse.bass_utils import run_bass_kernel_spmd

    nc = _get_module()
    in_maps = _host_prep(x, mask, qkv_w, qkv_b, out_w)
    last = None
    for attempt in range(3):
        try:
            res = run_bass_kernel_spmd(nc, in_maps, core_ids=list(range(N_CORES)))
            return _host_gather(res.results, qkv_b, out_b, out_w)
        except Exception as e:  # rare transient device faults: retry after recovery
            last = e
            time.sleep(10 * (attempt + 1))
    raise last
